# revision 4
# baseline (speedup 1.0000x reference)
"""Trainium2 Bass kernel for the AttentionCritic problem — V4.

V3 + software pipelining to keep the PE at full clock:
  - logits for chunk 0 are computed DURING the dense phase (pr products
    issued as soon as both agents' K/Q are ready; reduces trail by 2)
  - numerator/critic of chunk c is zipped with the logits phase of
    chunk c+1 (PE gets selbc/wc1b/reduce back-to-back; DVE alternates
    prod and pr; Scalar does copies; Pool absorbs 1/3 of prod TTs)
  - prod multiply path is split 3 ways (direct-PSUM DVE /
    Scalar-copy+Pool-TT / Scalar-copy+DVE-TT), pattern per phase
  - dense PSUM rotates over 3 pair-tiles, logits pair reserved
"""

import sys

sys.path.insert(0, "/opt/trn_rl_repo")

import numpy as np
import ml_dtypes

N, B, SDIM, ADIM, HID, HEADS = 8, 32768, 128, 16, 128, 4
AD = HID // HEADS
IDIM = SDIM + ADIM
NCORES = 8
BSH = B // NCORES
BF16 = ml_dtypes.bfloat16
INV_SQRT_AD = 1.0 / np.sqrt(AD).astype(np.float32)

OFF_WE1 = 0
OFF_WS = OFF_WE1 + 8 * 128
OFF_WC1A = OFF_WS + 8 * 128
OFF_WC1B = OFF_WC1A + 8 * 128
OFF_WK = OFF_WC1B + 8 * 128
OFF_WQ = OFF_WK + 128
OFF_WV = OFF_WQ + 128
OFF_SELBC = OFF_WV + 128
OFF_ONES = OFF_SELBC + 8 * 128
OFF_SELDD = OFF_ONES + 8 * 32
OFF_SELREP = OFF_SELDD + 2 * 32
OFF_WC2 = OFF_SELREP + 2 * 128
NBW = OFF_WC2 + 8 * 16
FB_BE = 0
FB_BS = 8
FB_BV = 16
FB_BC1 = 17
NBF = 25


def build_nc(bsh, split=True):
    import concourse.bass as bass
    import concourse.mybir as mybir
    from concourse.tile import TileContext

    f32 = mybir.dt.float32
    bf16 = mybir.dt.bfloat16
    MULT = mybir.AluOpType.mult
    COPY = mybir.ActivationFunctionType.Copy
    RELU = mybir.ActivationFunctionType.Relu
    EXP = mybir.ActivationFunctionType.Exp
    LN = mybir.ActivationFunctionType.Ln

    SC = min(1024, bsh)
    CS = min(512, SC)
    n_sc = bsh // SC
    n_cs = SC // CS

    nc = bass.Bass()
    dp = nc.declare_dram_parameter
    sT = dp("sT", [N, SDIM, bsh], bf16, isOutput=False)
    aT = dp("aT", [N, ADIM, bsh], bf16, isOutput=False)
    wblob = dp("wblob", [128, NBW], bf16, isOutput=False)
    we2b = dp("we2b", [ADIM, 8 * 128], bf16, isOutput=False)
    fblob = dp("fblob", [128, NBF], f32, isOutput=False)
    q8T = dp("q8T", [ADIM, N, bsh], f32, isOutput=True)

    with TileContext(nc) as tc:
        with (
            tc.tile_pool(name="const", bufs=1) as cp,
            tc.tile_pool(name="inw", bufs=2) as iw,
            tc.tile_pool(name="persist", bufs=1) as pp,
            tc.tile_pool(name="wk", bufs=2) as wp,
            tc.tile_pool(name="prp", bufs=8) as prp,
            tc.tile_pool(name="scl", bufs=2) as sp,
            tc.tile_pool(name="psum", bufs=1, space="PSUM") as qp,
        ):

            def load_inputs(sc):
                stc = iw.tile([SDIM, N * SC], bf16, tag="stc", name="stc")
                atc = iw.tile([ADIM, N * SC], bf16, tag="atc", name="atc")
                for n in range(N):
                    nc.sync.dma_start(out=stc[:, n * SC : (n + 1) * SC], in_=sT[n, :, sc * SC : (sc + 1) * SC])
                    nc.sync.dma_start(out=atc[:, n * SC : (n + 1) * SC], in_=aT[n, :, sc * SC : (sc + 1) * SC])
                return stc, atc

            ins0 = load_inputs(0)
            W = cp.tile([128, NBW], bf16, tag="wblob", name="wblob")
            nc.sync.dma_start(out=W[:], in_=wblob[:])
            W2 = cp.tile([ADIM, 8 * 128], bf16, tag="we2b", name="we2b")
            nc.sync.dma_start(out=W2[:], in_=we2b[:])
            F = cp.tile([128, NBF], f32, tag="fblob", name="fblob")
            nc.sync.dma_start(out=F[:], in_=fblob[:])

            def persist(tag):
                return pp.tile([HID, N * SC], bf16, tag=tag, name=tag)

            def pair(tag):
                return qp.tile([HID, 2 * CS], f32, tag=tag, name=tag)

            state = {}

            # ---------- P1 (logits) issue helpers ----------
            def make_pr(i, j, c):
                Qt, Kt = state["Qt"], state["Kt"]
                c0 = c * CS
                pr = prp.tile([HID, CS], bf16, tag="pr", name="pr")
                nc.vector.tensor_tensor(
                    pr[:], Qt[:, i * SC + c0 : i * SC + c0 + CS],
                    Kt[:, j * SC + c0 : j * SC + c0 + CS], MULT)
                return pr

            def make_reduce(i, j, pr, Lp, first, last):
                g = i % 4
                nc.tensor.matmul(
                    Lp[32 * g : 32 * g + 32, (i // 4) * CS : (i // 4) * CS + CS],
                    W[:, OFF_ONES + 32 * j : OFF_ONES + 32 * j + 32],
                    pr[:], start=first, stop=last,
                    tile_position=(0, 32 * g), skip_group_check=True)

            def p1_stream(c, Lp):
                """Yield thunk pairs (pr_thunk, reduce_thunk) for all 56
                pairs of chunk c in j-outer order."""
                cnt = [0] * N
                for j in range(N):
                    for i in range(N):
                        if i == j:
                            continue
                        def mk(i=i, j=j):
                            first = cnt[i] == 0
                            last = cnt[i] == 6
                            cnt[i] += 1
                            pr = make_pr(i, j, c)
                            return lambda pr=pr, first=first, last=last: make_reduce(i, j, pr, Lp, first, last)
                        yield mk

            # ---------- softmax helper ----------
            def softmax(c, Lp, Dpair):
                Et = wp.tile([HID, 2 * CS], bf16, tag="Et", name="Et")
                nc.scalar.activation(Et[:], Lp[:], EXP, scale=float(INV_SQRT_AD))
                for b in range(2):
                    nc.tensor.matmul(
                        Dpair[0:32, 0:CS],
                        W[:, OFF_SELDD + 32 * b : OFF_SELDD + 32 * b + 32],
                        Et[:, b * CS : (b + 1) * CS],
                        start=(b == 0), stop=(b == 1))
                lnD = sp.tile([32, CS], f32, tag="lnD", name="lnD")
                nc.scalar.activation(lnD[:], Dpair[0:32, 0:CS], LN)
                minv = sp.tile([32, CS], bf16, tag="minv", name="minv")
                nc.scalar.activation(minv[:], lnD[:], EXP, scale=-1.0)
                for b in range(2):
                    nc.tensor.matmul(
                        Dpair[:, b * CS : (b + 1) * CS],
                        W[0:32, OFF_SELREP + 128 * b : OFF_SELREP + 128 * b + 128],
                        minv[:], start=True, stop=True)
                At = wp.tile([HID, 2 * CS], bf16, tag="At", name="At")
                nc.vector.tensor_tensor(At[:], Et[:], Dpair[:], MULT)
                return At

            # ---------- numerator/critic issue stream ----------
            def numer_stream(c, At, scr_tags, pattern):
                """Yield thunks; each issues one pipeline step (selbc+prod+
                wc1b) or a quarter boundary (wc1a / h1+wc2+aq+dma)."""
                senc_all, Vt = state["senc"], state["Vt"]
                sc = state["sc"]
                c0 = c * CS
                apg = 2 if CS == 512 else 1
                mcnt = [0]
                n_scr = len(scr_tags) * 2
                for q in range(N // apg):
                    agents = [apg * q + t for t in range(apg)]
                    hp = pair("Ha" if q % 2 == 0 else "Hb")

                    def start_q(agents=agents, hp=hp):
                        for ii, i in enumerate(agents):
                            nc.tensor.matmul(
                                hp[:, ii * CS : ii * CS + CS],
                                W[:, OFF_WC1A + i * 128 : OFF_WC1A + i * 128 + 128],
                                senc_all[:, i * SC + c0 : i * SC + c0 + CS],
                                start=True, stop=False)
                    yield start_q
                    for j in range(N):
                        for ii, i in enumerate(agents):
                            if i == j:
                                continue

                            def step(i=i, ii=ii, j=j, hp=hp):
                                g = i % 4
                                m = mcnt[0]
                                sidx = m % n_scr
                                scr = pair(scr_tags[sidx // 2])
                                half = sidx % 2
                                nc.tensor.matmul(
                                    scr[:, half * CS : half * CS + CS],
                                    W[32 * g : 32 * g + 32, OFF_SELBC + j * 128 : OFF_SELBC + j * 128 + 128],
                                    At[32 * g : 32 * g + 32, (i // 4) * CS : (i // 4) * CS + CS],
                                    start=True, stop=True,
                                    tile_position=(32 * g, 0))
                                prod = wp.tile([HID, CS], bf16, tag=f"prod{m % 4}", name="prod")
                                path = pattern[m % len(pattern)]
                                vsl = Vt[:, j * SC + c0 : j * SC + c0 + CS]
                                if path == "d":
                                    nc.vector.tensor_tensor(prod[:], scr[:, half * CS : half * CS + CS], vsl, MULT)
                                else:
                                    ebs = wp.tile([HID, CS], bf16, tag=f"ebs{m % 2}", name="ebs")
                                    nc.scalar.activation(ebs[:], scr[:, half * CS : half * CS + CS], COPY)
                                    eng = nc.gpsimd if path == "p" else nc.vector
                                    eng.tensor_tensor(prod[:], ebs[:], vsl, MULT)
                                mcnt[0] += 1
                                last_j = N - 1 if i != N - 1 else N - 2
                                nc.tensor.matmul(
                                    hp[:, ii * CS : ii * CS + CS],
                                    W[:, OFF_WC1B + i * 128 : OFF_WC1B + i * 128 + 128],
                                    prod[:], start=False, stop=(j == last_j))
                            yield step

                    def end_q(agents=agents, hp=hp, q=q):
                        h1 = wp.tile([HID, apg * CS], bf16, tag="h1", name="h1")
                        for ii, i in enumerate(agents):
                            nc.scalar.activation(
                                h1[:, ii * CS : ii * CS + CS], hp[:, ii * CS : ii * CS + CS],
                                RELU, bias=F[:, FB_BC1 + i : FB_BC1 + i + 1])
                        for ii, i in enumerate(agents):
                            nc.tensor.matmul(
                                hp[0:ADIM, ii * CS : ii * CS + CS],
                                W[:, OFF_WC2 + i * 16 : OFF_WC2 + i * 16 + 16],
                                h1[:, ii * CS : ii * CS + CS], start=True, stop=True,
                                skip_group_check=True)
                        aq = sp.tile([ADIM, apg * CS], f32, tag="aq", name="aq")
                        nc.scalar.activation(aq[:], hp[0:ADIM, 0 : apg * CS], COPY)
                        nc.sync.dma_start(
                            out=q8T[:, apg * q : apg * q + apg, sc * SC + c0 : sc * SC + c0 + CS],
                            in_=aq[:])
                    yield end_q

            # ================= main loop =================
            for sc in range(n_sc):
                stc, atc = ins0 if sc == 0 else load_inputs(sc)
                senc_all = persist("senc")
                Kt = persist("Kt")
                Qt = persist("Qt")
                Vt = persist("Vt")
                state.update(senc=senc_all, Kt=Kt, Qt=Qt, Vt=Vt, sc=sc)

                # ---- dense phase with embedded P1(c=0) ----
                Lp0 = pair("Lp")
                rot = ["Sp", "Ha", "Hb"]
                ucnt = [0]

                def dpair():
                    t = pair(rot[ucnt[0] % 3])
                    ucnt[0] += 1
                    return t

                # P1(c0) pair order by agent availability
                cnt0 = [0] * N
                pend_red = []

                def issue_pr0(i, j):
                    first = cnt0[i] == 0
                    last = cnt0[i] == 6
                    cnt0[i] += 1
                    pr = make_pr(i, j, 0)
                    pend_red.append(lambda pr=pr, i=i, j=j, first=first, last=last: make_reduce(i, j, pr, Lp0, first, last))

                red_done = [0]

                def drain_red(upto):
                    while red_done[0] < min(upto, len(pend_red)):
                        pend_red[red_done[0]]()
                        red_done[0] += 1

                npr = [0]
                for n in range(N):
                    nsl = slice(n * SC, (n + 1) * SC)
                    pe_ = dpair()
                    po_ = dpair()
                    for h in range(n_cs):
                        hs = slice(h * CS, (h + 1) * CS)
                        nsl_h = slice(n * SC + h * CS, n * SC + (h + 1) * CS)
                        nc.tensor.matmul(
                            pe_[:, hs], W[:, OFF_WE1 + n * 128 : OFF_WE1 + n * 128 + 128],
                            stc[:, nsl_h], start=True, stop=False)
                        nc.tensor.matmul(
                            pe_[:, hs], W2[:, n * 128 : n * 128 + 128],
                            atc[:, nsl_h], start=False, stop=True)
                        nc.tensor.matmul(
                            po_[:, hs], W[:, OFF_WS + n * 128 : OFF_WS + n * 128 + 128],
                            stc[:, nsl_h], start=True, stop=True)
                    saenc = wp.tile([HID, SC], bf16, tag="saenc", name="saenc")
                    nc.scalar.activation(saenc[:], pe_[:, :SC], RELU, bias=F[:, FB_BE + n : FB_BE + n + 1])
                    nc.scalar.activation(senc_all[:, nsl], po_[:, :SC], RELU, bias=F[:, FB_BS + n : FB_BS + n + 1])
                    pk = dpair()
                    pq = dpair()
                    pv = dpair()
                    for h in range(n_cs):
                        hs = slice(h * CS, (h + 1) * CS)
                        nc.tensor.matmul(pk[:, hs], W[:, OFF_WK : OFF_WK + 128], saenc[:, hs], start=True, stop=True)
                        nc.tensor.matmul(pq[:, hs], W[:, OFF_WQ : OFF_WQ + 128],
                                         senc_all[:, n * SC + h * CS : n * SC + (h + 1) * CS], start=True, stop=True)
                        nc.tensor.matmul(pv[:, hs], W[:, OFF_WV : OFF_WV + 128], saenc[:, hs], start=True, stop=True)
                    nc.vector.tensor_copy(Kt[:, nsl], pk[:, :SC])
                    nc.vector.tensor_copy(Qt[:, nsl], pq[:, :SC])
                    nc.scalar.activation(Vt[:, nsl], pv[:, :SC], RELU, bias=F[:, FB_BV : FB_BV + 1])
                    # issue newly-available pr products (both agents <= n done)
                    for i in range(n):
                        issue_pr0(i, n)
                        npr[0] += 1
                        drain_red(npr[0] - 2)
                        issue_pr0(n, i)
                        npr[0] += 1
                        drain_red(npr[0] - 2)
                drain_red(len(pend_red))

                # ---- softmax(c0) ----
                At0 = softmax(0, Lp0, pair("Sp"))

                if n_cs == 2:
                    # ---- zip: numer(c0) || P1(c1) ----
                    Lp1 = pair("Lp")
                    p1 = list(p1_stream(1, Lp1))
                    num = list(numer_stream(0, At0, ["Sp"], pattern="dps"))
                    pend = []
                    drained = [0]

                    def drain(upto):
                        while drained[0] < min(upto, len(pend)):
                            pend[drained[0]]()
                            drained[0] += 1

                    pi = 0
                    for k in range(max(len(num), len(p1))):
                        if pi < len(p1):
                            pend.append(p1[pi]())  # issues pr, returns reduce thunk
                            pi += 1
                        drain(k - 1)
                        if k < len(num):
                            num[k]()
                    drain(len(pend))
                    # ---- softmax(c1) + numer(c1) solo ----
                    At1 = softmax(1, Lp1, pair("Sp"))
                    for t in numer_stream(1, At1, ["Lp", "Sp"], pattern="dpds"):
                        t()
                else:
                    for t in numer_stream(0, At0, ["Lp", "Sp"], pattern="dpds"):
                        t()
    if split:
        split_multi_waits(nc)
    return nc


def split_multi_waits(nc):
    """Hoist all but one sync-wait of each instruction onto same-engine NoOps
    (the 64B ISA instruction structs carry exactly one wait slot)."""
    import concourse.mybir as mybir

    nid = [0]
    for f in nc.m.functions:
        for blk in f.blocks:
            il = blk.instructions
            i = 0
            while i < len(il):
                inst = il[i]
                si = inst.sync_info
                if si is not None and si.on_wait and len(si.on_wait) > 1:
                    waits = list(si.on_wait)
                    extra, keep = waits[:-1], waits[-1:]
                    si.on_wait = keep
                    for w in extra:
                        nid[0] += 1
                        nop = mybir.InstNoOp(name=f"W-split-{nid[0]}", ins=[], outs=[])
                        nop.engine = inst.engine
                        nop.sync_info = mybir.SyncInfo(on_wait=[w], on_update=[])
                        il.insert(i, nop)
                        i += 1
                i += 1
    return nc


def host_prep(states, actions, We, be, Ws, bs, Wk, Wq, Wv, bv, Wc1, bc1, Wc2, bc2):
    f32 = np.float32

    def bf(x):
        return np.ascontiguousarray(x, dtype=BF16)

    acs = np.argmax(actions, axis=-1)  # [N, B]

    wk_m = np.concatenate([Wk[k] for k in range(HEADS)], axis=1)
    wq_m = np.concatenate([Wq[k] for k in range(HEADS)], axis=1)
    wv_m = np.concatenate([Wv[k] for k in range(HEADS)], axis=1)
    bv_m = np.concatenate([bv[k] for k in range(HEADS)], axis=0)

    onesred = np.zeros((128, 8 * 32), f32)
    for j in range(N):
        for k in range(HEADS):
            onesred[32 * k : 32 * (k + 1), 32 * j + 4 * j + k] = 1.0
    selbc = np.zeros((128, 8 * 128), f32)
    for g in range(4):
        for j in range(N):
            for k in range(HEADS):
                selbc[32 * g + 4 * j + k, 128 * j + 32 * k : 128 * j + 32 * (k + 1)] = 1.0
    seldd = np.zeros((128, 2 * 32), f32)
    for b2 in range(2):
        for g in range(4):
            for k in range(HEADS):
                for j in range(N):
                    if j != 4 * b2 + g:
                        seldd[32 * g + 4 * j + k, 32 * b2 + 16 * b2 + 4 * g + k] = 1.0
    selrep = np.zeros((128, 2 * 128), f32)
    for b2 in range(2):
        for g in range(4):
            for k in range(HEADS):
                for j in range(N):
                    selrep[16 * b2 + 4 * g + k, 128 * b2 + 32 * g + 4 * j + k] = 1.0

    wb = np.zeros((128, NBW), f32)
    for n in range(N):
        wb[:, OFF_WE1 + n * 128 : OFF_WE1 + (n + 1) * 128] = We[n, :SDIM, :]
        wb[:, OFF_WS + n * 128 : OFF_WS + (n + 1) * 128] = Ws[n]
        wb[:, OFF_WC1A + n * 128 : OFF_WC1A + (n + 1) * 128] = Wc1[n, :HID, :]
        wb[:, OFF_WC1B + n * 128 : OFF_WC1B + (n + 1) * 128] = Wc1[n, HID:, :]
        wb[:, OFF_WC2 + n * 16 : OFF_WC2 + (n + 1) * 16] = Wc2[n]
    wb[:, OFF_WK : OFF_WK + 128] = wk_m
    wb[:, OFF_WQ : OFF_WQ + 128] = wq_m
    wb[:, OFF_WV : OFF_WV + 128] = wv_m
    wb[:, OFF_SELBC : OFF_SELBC + 1024] = selbc
    wb[:, OFF_ONES : OFF_ONES + 256] = onesred
    wb[:, OFF_SELDD : OFF_SELDD + 64] = seldd
    wb[:, OFF_SELREP : OFF_SELREP + 256] = selrep

    we2blob = np.zeros((ADIM, 8 * 128), f32)
    for n in range(N):
        we2blob[:, n * 128 : (n + 1) * 128] = We[n, SDIM:, :]

    fb = np.zeros((128, NBF), f32)
    fb[:, FB_BE : FB_BE + 8] = be.T
    fb[:, FB_BS : FB_BS + 8] = bs.T
    fb[:, FB_BV] = bv_m
    fb[:, FB_BC1 : FB_BC1 + 8] = bc1.T

    shared = {"wblob": bf(wb), "we2b": bf(we2blob), "fblob": np.ascontiguousarray(fb, f32)}
    sT_full = bf(states.transpose(0, 2, 1))
    aT_full = bf(actions.transpose(0, 2, 1))

    def core_inputs(c, bsh):
        lo = c * bsh
        return dict(
            shared,
            sT=np.ascontiguousarray(sT_full[:, :, lo : lo + bsh]),
            aT=np.ascontiguousarray(aT_full[:, :, lo : lo + bsh]),
        )

    return core_inputs, acs, np.asarray(bc2, np.float32)


def kernel(**inputs):
    from concourse.bass_utils import run_bass_kernel_spmd

    nc = build_nc(BSH)
    core_inputs, acs, bc2 = host_prep(**inputs)
    in_maps = [core_inputs(c, BSH) for c in range(NCORES)]
    res = run_bass_kernel_spmd(nc, in_maps, list(range(NCORES))).results
    out = np.empty((N, B, 1), np.float32)
    for c in range(NCORES):
        aqT = res[c]["q8T"]  # [16, N, BSH]
        aq = np.ascontiguousarray(aqT.transpose(1, 0, 2))
        sl = slice(c * BSH, (c + 1) * BSH)
        picked = np.take_along_axis(aq, acs[:, None, sl], axis=1)[:, 0, :]
        out[:, sl, 0] = picked + np.take_along_axis(bc2, acs[:, sl], axis=1)
    return out


# revision 5
# speedup vs baseline: 1.0597x; 1.0597x over previous
"""Trainium2 Bass kernel for the AttentionCritic problem — V4.

V3 + software pipelining to keep the PE at full clock:
  - logits for chunk 0 are computed DURING the dense phase (pr products
    issued as soon as both agents' K/Q are ready; reduces trail by 2)
  - numerator/critic of chunk c is zipped with the logits phase of
    chunk c+1 (PE gets selbc/wc1b/reduce back-to-back; DVE alternates
    prod and pr; Scalar does copies; Pool absorbs 1/3 of prod TTs)
  - prod multiply path is split 3 ways (direct-PSUM DVE /
    Scalar-copy+Pool-TT / Scalar-copy+DVE-TT), pattern per phase
  - dense PSUM rotates over 3 pair-tiles, logits pair reserved
"""

import sys

sys.path.insert(0, "/opt/trn_rl_repo")

import numpy as np
import ml_dtypes

N, B, SDIM, ADIM, HID, HEADS = 8, 32768, 128, 16, 128, 4
AD = HID // HEADS
IDIM = SDIM + ADIM
NCORES = 8
BSH = B // NCORES
BF16 = ml_dtypes.bfloat16
INV_SQRT_AD = 1.0 / np.sqrt(AD).astype(np.float32)

OFF_WE1 = 0
OFF_WS = OFF_WE1 + 8 * 128
OFF_WC1A = OFF_WS + 8 * 128
OFF_WC1B = OFF_WC1A + 8 * 128
OFF_WK = OFF_WC1B + 8 * 128
OFF_WQ = OFF_WK + 128
OFF_WV = OFF_WQ + 128
OFF_SELBC = OFF_WV + 128
OFF_ONES = OFF_SELBC + 8 * 128
OFF_SELDD = OFF_ONES + 8 * 32
OFF_SELREP = OFF_SELDD + 2 * 32
OFF_WC2 = OFF_SELREP + 2 * 128
NBW = OFF_WC2 + 8 * 16
FB_BE = 0
FB_BS = 8
FB_BV = 16
FB_BC1 = 17
NBF = 25


def build_nc(bsh, split=True):
    import concourse.bass as bass
    import concourse.mybir as mybir
    from concourse.tile import TileContext

    f32 = mybir.dt.float32
    bf16 = mybir.dt.bfloat16
    MULT = mybir.AluOpType.mult
    COPY = mybir.ActivationFunctionType.Copy
    RELU = mybir.ActivationFunctionType.Relu
    EXP = mybir.ActivationFunctionType.Exp
    LN = mybir.ActivationFunctionType.Ln

    SC = min(1024, bsh)
    CS = min(512, SC)
    n_sc = bsh // SC
    n_cs = SC // CS

    nc = bass.Bass()
    dp = nc.declare_dram_parameter
    sT = dp("sT", [N, SDIM, bsh], bf16, isOutput=False)
    aT = dp("aT", [N, ADIM, bsh], bf16, isOutput=False)
    wblob = dp("wblob", [128, NBW], bf16, isOutput=False)
    we2b = dp("we2b", [ADIM, 8 * 128], bf16, isOutput=False)
    fblob = dp("fblob", [128, NBF], f32, isOutput=False)
    q8T = dp("q8T", [ADIM, N, bsh], f32, isOutput=True)

    with TileContext(nc) as tc:
        with (
            tc.tile_pool(name="const", bufs=1) as cp,
            tc.tile_pool(name="inw", bufs=2) as iw,
            tc.tile_pool(name="persist", bufs=1) as pp,
            tc.tile_pool(name="wk", bufs=2) as wp,
            tc.tile_pool(name="prp", bufs=8) as prp,
            tc.tile_pool(name="scl", bufs=2) as sp,
            tc.tile_pool(name="psum", bufs=1, space="PSUM") as qp,
        ):

            def load_inputs(sc):
                stc = iw.tile([SDIM, N * SC], bf16, tag="stc", name="stc")
                atc = iw.tile([ADIM, N * SC], bf16, tag="atc", name="atc")
                for n in range(N):
                    nc.sync.dma_start(out=stc[:, n * SC : (n + 1) * SC], in_=sT[n, :, sc * SC : (sc + 1) * SC])
                    nc.sync.dma_start(out=atc[:, n * SC : (n + 1) * SC], in_=aT[n, :, sc * SC : (sc + 1) * SC])
                return stc, atc

            ins0 = load_inputs(0)
            W = cp.tile([128, NBW], bf16, tag="wblob", name="wblob")
            nc.sync.dma_start(out=W[:], in_=wblob[:])
            W2 = cp.tile([ADIM, 8 * 128], bf16, tag="we2b", name="we2b")
            nc.sync.dma_start(out=W2[:], in_=we2b[:])
            F = cp.tile([128, NBF], f32, tag="fblob", name="fblob")
            nc.sync.dma_start(out=F[:], in_=fblob[:])

            def persist(tag):
                return pp.tile([HID, N * SC], bf16, tag=tag, name=tag)

            def pair(tag):
                return qp.tile([HID, 2 * CS], f32, tag=tag, name=tag)

            state = {}

            # ---------- P1 (logits) issue helpers ----------
            def make_pr(i, j, c):
                Qt, Kt = state["Qt"], state["Kt"]
                c0 = c * CS
                pr = prp.tile([HID, CS], bf16, tag="pr", name="pr")
                nc.vector.tensor_tensor(
                    pr[:], Qt[:, i * SC + c0 : i * SC + c0 + CS],
                    Kt[:, j * SC + c0 : j * SC + c0 + CS], MULT)
                return pr

            def make_reduce(i, j, pr, Lp, first, last):
                g = i % 4
                nc.tensor.matmul(
                    Lp[32 * g : 32 * g + 32, (i // 4) * CS : (i // 4) * CS + CS],
                    W[:, OFF_ONES + 32 * j : OFF_ONES + 32 * j + 32],
                    pr[:], start=first, stop=last,
                    tile_position=(0, 32 * g), skip_group_check=True)

            def p1_stream(c, Lp):
                """Yield thunk pairs (pr_thunk, reduce_thunk) for all 56
                pairs of chunk c in j-outer order."""
                cnt = [0] * N
                for j in range(N):
                    for i in range(N):
                        if i == j:
                            continue
                        def mk(i=i, j=j):
                            first = cnt[i] == 0
                            last = cnt[i] == 6
                            cnt[i] += 1
                            pr = make_pr(i, j, c)
                            return lambda pr=pr, first=first, last=last: make_reduce(i, j, pr, Lp, first, last)
                        yield mk

            # ---------- softmax helper ----------
            def softmax(c, Lp, Dpair):
                Et = wp.tile([HID, 2 * CS], bf16, tag="Et", name="Et")
                nc.scalar.activation(Et[:], Lp[:], EXP, scale=float(INV_SQRT_AD))
                for b in range(2):
                    nc.tensor.matmul(
                        Dpair[0:32, 0:CS],
                        W[:, OFF_SELDD + 32 * b : OFF_SELDD + 32 * b + 32],
                        Et[:, b * CS : (b + 1) * CS],
                        start=(b == 0), stop=(b == 1))
                lnD = sp.tile([32, CS], f32, tag="lnD", name="lnD")
                nc.scalar.activation(lnD[:], Dpair[0:32, 0:CS], LN)
                minv = sp.tile([32, CS], bf16, tag="minv", name="minv")
                nc.scalar.activation(minv[:], lnD[:], EXP, scale=-1.0)
                for b in range(2):
                    nc.tensor.matmul(
                        Dpair[:, b * CS : (b + 1) * CS],
                        W[0:32, OFF_SELREP + 128 * b : OFF_SELREP + 128 * b + 128],
                        minv[:], start=True, stop=True)
                At = wp.tile([HID, 2 * CS], bf16, tag="At", name="At")
                nc.vector.tensor_tensor(At[:], Et[:], Dpair[:], MULT)
                return At

            # ---------- numerator/critic issue stream ----------
            def numer_stream(c, At, scr_tags, pattern, lag=6):
                """Yield thunks; selbc+prod issue at step k, the matching
                wc1b trails by `lag` steps so the PE never waits on a
                just-produced prod."""
                senc_all, Vt = state["senc"], state["Vt"]
                sc = state["sc"]
                c0 = c * CS
                apg = 2 if CS == 512 else 1
                mcnt = [0]
                n_scr = len(scr_tags) * 2
                wcq = []

                def flush_wcq(keep):
                    while len(wcq) > keep:
                        wcq.pop(0)()

                for q in range(N // apg):
                    agents = [apg * q + t for t in range(apg)]
                    hp = pair("Ha" if q % 2 == 0 else "Hb")

                    def start_q(agents=agents, hp=hp):
                        for ii, i in enumerate(agents):
                            nc.tensor.matmul(
                                hp[:, ii * CS : ii * CS + CS],
                                W[:, OFF_WC1A + i * 128 : OFF_WC1A + i * 128 + 128],
                                senc_all[:, i * SC + c0 : i * SC + c0 + CS],
                                start=True, stop=False)
                    yield start_q
                    for j in range(N):
                        for ii, i in enumerate(agents):
                            if i == j:
                                continue

                            def step(i=i, ii=ii, j=j, hp=hp):
                                g = i % 4
                                m = mcnt[0]
                                sidx = m % n_scr
                                scr = pair(scr_tags[sidx // 2])
                                half = sidx % 2
                                nc.tensor.matmul(
                                    scr[:, half * CS : half * CS + CS],
                                    W[32 * g : 32 * g + 32, OFF_SELBC + j * 128 : OFF_SELBC + j * 128 + 128],
                                    At[32 * g : 32 * g + 32, (i // 4) * CS : (i // 4) * CS + CS],
                                    start=True, stop=True,
                                    tile_position=(32 * g, 0))
                                prod = wp.tile([HID, CS], bf16, tag=f"prod{m % 8}", name="prod")
                                path = pattern[m % len(pattern)]
                                vsl = Vt[:, j * SC + c0 : j * SC + c0 + CS]
                                if path == "d":
                                    nc.vector.tensor_tensor(prod[:], scr[:, half * CS : half * CS + CS], vsl, MULT)
                                else:
                                    ebs = wp.tile([HID, CS], bf16, tag=f"ebs{m % 2}", name="ebs")
                                    nc.scalar.activation(ebs[:], scr[:, half * CS : half * CS + CS], COPY)
                                    eng = nc.gpsimd if path == "p" else nc.vector
                                    eng.tensor_tensor(prod[:], ebs[:], vsl, MULT)
                                mcnt[0] += 1
                                last_j = N - 1 if i != N - 1 else N - 2

                                def wc1b(prod=prod, i=i, ii=ii, j=j, hp=hp, stop=(j == last_j)):
                                    nc.tensor.matmul(
                                        hp[:, ii * CS : ii * CS + CS],
                                        W[:, OFF_WC1B + i * 128 : OFF_WC1B + i * 128 + 128],
                                        prod[:], start=False, stop=stop)
                                wcq.append(wc1b)
                                flush_wcq(lag)
                            yield step

                    def end_q(agents=agents, hp=hp, q=q):
                        flush_wcq(0)
                        h1 = wp.tile([HID, apg * CS], bf16, tag="h1", name="h1")
                        for ii, i in enumerate(agents):
                            nc.scalar.activation(
                                h1[:, ii * CS : ii * CS + CS], hp[:, ii * CS : ii * CS + CS],
                                RELU, bias=F[:, FB_BC1 + i : FB_BC1 + i + 1])
                        for ii, i in enumerate(agents):
                            nc.tensor.matmul(
                                hp[0:ADIM, ii * CS : ii * CS + CS],
                                W[:, OFF_WC2 + i * 16 : OFF_WC2 + i * 16 + 16],
                                h1[:, ii * CS : ii * CS + CS], start=True, stop=True,
                                skip_group_check=True)
                        aq = sp.tile([ADIM, apg * CS], f32, tag="aq", name="aq")
                        nc.scalar.activation(aq[:], hp[0:ADIM, 0 : apg * CS], COPY)
                        nc.sync.dma_start(
                            out=q8T[:, apg * q : apg * q + apg, sc * SC + c0 : sc * SC + c0 + CS],
                            in_=aq[:])
                    yield end_q

            # ================= main loop =================
            for sc in range(n_sc):
                stc, atc = ins0 if sc == 0 else load_inputs(sc)
                senc_all = persist("senc")
                Kt = persist("Kt")
                Qt = persist("Qt")
                Vt = persist("Vt")
                state.update(senc=senc_all, Kt=Kt, Qt=Qt, Vt=Vt, sc=sc)

                # ---- dense phase with embedded P1(c=0) ----
                Lp0 = pair("Lp")
                rot = ["Sp", "Ha", "Hb"]
                ucnt = [0]

                def dpair():
                    t = pair(rot[ucnt[0] % 3])
                    ucnt[0] += 1
                    return t

                # P1(c0) pair order by agent availability
                cnt0 = [0] * N
                pend_red = []

                def issue_pr0(i, j):
                    first = cnt0[i] == 0
                    last = cnt0[i] == 6
                    cnt0[i] += 1
                    pr = make_pr(i, j, 0)
                    pend_red.append(lambda pr=pr, i=i, j=j, first=first, last=last: make_reduce(i, j, pr, Lp0, first, last))

                red_done = [0]

                def drain_red(upto):
                    while red_done[0] < min(upto, len(pend_red)):
                        pend_red[red_done[0]]()
                        red_done[0] += 1

                npr = [0]
                for n in range(N):
                    nsl = slice(n * SC, (n + 1) * SC)
                    pe_ = dpair()
                    po_ = dpair()
                    for h in range(n_cs):
                        hs = slice(h * CS, (h + 1) * CS)
                        nsl_h = slice(n * SC + h * CS, n * SC + (h + 1) * CS)
                        nc.tensor.matmul(
                            pe_[:, hs], W[:, OFF_WE1 + n * 128 : OFF_WE1 + n * 128 + 128],
                            stc[:, nsl_h], start=True, stop=False)
                        nc.tensor.matmul(
                            pe_[:, hs], W2[:, n * 128 : n * 128 + 128],
                            atc[:, nsl_h], start=False, stop=True)
                        nc.tensor.matmul(
                            po_[:, hs], W[:, OFF_WS + n * 128 : OFF_WS + n * 128 + 128],
                            stc[:, nsl_h], start=True, stop=True)
                    saenc = wp.tile([HID, SC], bf16, tag="saenc", name="saenc")
                    nc.scalar.activation(saenc[:], pe_[:, :SC], RELU, bias=F[:, FB_BE + n : FB_BE + n + 1])
                    nc.scalar.activation(senc_all[:, nsl], po_[:, :SC], RELU, bias=F[:, FB_BS + n : FB_BS + n + 1])
                    pk = dpair()
                    pq = dpair()
                    pv = dpair()
                    for h in range(n_cs):
                        hs = slice(h * CS, (h + 1) * CS)
                        nc.tensor.matmul(pk[:, hs], W[:, OFF_WK : OFF_WK + 128], saenc[:, hs], start=True, stop=True)
                        nc.tensor.matmul(pq[:, hs], W[:, OFF_WQ : OFF_WQ + 128],
                                         senc_all[:, n * SC + h * CS : n * SC + (h + 1) * CS], start=True, stop=True)
                        nc.tensor.matmul(pv[:, hs], W[:, OFF_WV : OFF_WV + 128], saenc[:, hs], start=True, stop=True)
                    nc.vector.tensor_copy(Kt[:, nsl], pk[:, :SC])
                    nc.vector.tensor_copy(Qt[:, nsl], pq[:, :SC])
                    nc.scalar.activation(Vt[:, nsl], pv[:, :SC], RELU, bias=F[:, FB_BV : FB_BV + 1])
                    # issue newly-available pr products (both agents <= n done)
                    for i in range(n):
                        issue_pr0(i, n)
                        npr[0] += 1
                        drain_red(npr[0] - 6)
                        issue_pr0(n, i)
                        npr[0] += 1
                        drain_red(npr[0] - 6)
                drain_red(len(pend_red))

                # ---- softmax(c0) ----
                At0 = softmax(0, Lp0, pair("Sp"))

                if n_cs == 2:
                    # ---- zip: numer(c0) || P1(c1) ----
                    Lp1 = pair("Lp")
                    p1 = list(p1_stream(1, Lp1))
                    num = list(numer_stream(0, At0, ["Sp"], pattern="dssp"))
                    pend = []
                    drained = [0]

                    def drain(upto):
                        while drained[0] < min(upto, len(pend)):
                            pend[drained[0]]()
                            drained[0] += 1

                    pi = 0
                    for k in range(max(len(num), len(p1))):
                        if pi < len(p1):
                            pend.append(p1[pi]())  # issues pr, returns reduce thunk
                            pi += 1
                        drain(k - 5)
                        if k < len(num):
                            num[k]()
                    drain(len(pend))
                    # ---- softmax(c1) + numer(c1) solo ----
                    At1 = softmax(1, Lp1, pair("Sp"))
                    for t in numer_stream(1, At1, ["Lp", "Sp"], pattern="ddsp"):
                        t()
                else:
                    for t in numer_stream(0, At0, ["Lp", "Sp"], pattern="ddsp"):
                        t()
    if split:
        split_multi_waits(nc)
    return nc


def split_multi_waits(nc):
    """Hoist all but one sync-wait of each instruction onto same-engine NoOps
    (the 64B ISA instruction structs carry exactly one wait slot)."""
    import concourse.mybir as mybir

    nid = [0]
    for f in nc.m.functions:
        for blk in f.blocks:
            il = blk.instructions
            i = 0
            while i < len(il):
                inst = il[i]
                si = inst.sync_info
                if si is not None and si.on_wait and len(si.on_wait) > 1:
                    waits = list(si.on_wait)
                    extra, keep = waits[:-1], waits[-1:]
                    si.on_wait = keep
                    for w in extra:
                        nid[0] += 1
                        nop = mybir.InstNoOp(name=f"W-split-{nid[0]}", ins=[], outs=[])
                        nop.engine = inst.engine
                        nop.sync_info = mybir.SyncInfo(on_wait=[w], on_update=[])
                        il.insert(i, nop)
                        i += 1
                i += 1
    return nc


def host_prep(states, actions, We, be, Ws, bs, Wk, Wq, Wv, bv, Wc1, bc1, Wc2, bc2):
    f32 = np.float32

    def bf(x):
        return np.ascontiguousarray(x, dtype=BF16)

    acs = np.argmax(actions, axis=-1)  # [N, B]

    wk_m = np.concatenate([Wk[k] for k in range(HEADS)], axis=1)
    wq_m = np.concatenate([Wq[k] for k in range(HEADS)], axis=1)
    wv_m = np.concatenate([Wv[k] for k in range(HEADS)], axis=1)
    bv_m = np.concatenate([bv[k] for k in range(HEADS)], axis=0)

    onesred = np.zeros((128, 8 * 32), f32)
    for j in range(N):
        for k in range(HEADS):
            onesred[32 * k : 32 * (k + 1), 32 * j + 4 * j + k] = 1.0
    selbc = np.zeros((128, 8 * 128), f32)
    for g in range(4):
        for j in range(N):
            for k in range(HEADS):
                selbc[32 * g + 4 * j + k, 128 * j + 32 * k : 128 * j + 32 * (k + 1)] = 1.0
    seldd = np.zeros((128, 2 * 32), f32)
    for b2 in range(2):
        for g in range(4):
            for k in range(HEADS):
                for j in range(N):
                    if j != 4 * b2 + g:
                        seldd[32 * g + 4 * j + k, 32 * b2 + 16 * b2 + 4 * g + k] = 1.0
    selrep = np.zeros((128, 2 * 128), f32)
    for b2 in range(2):
        for g in range(4):
            for k in range(HEADS):
                for j in range(N):
                    selrep[16 * b2 + 4 * g + k, 128 * b2 + 32 * g + 4 * j + k] = 1.0

    wb = np.zeros((128, NBW), f32)
    for n in range(N):
        wb[:, OFF_WE1 + n * 128 : OFF_WE1 + (n + 1) * 128] = We[n, :SDIM, :]
        wb[:, OFF_WS + n * 128 : OFF_WS + (n + 1) * 128] = Ws[n]
        wb[:, OFF_WC1A + n * 128 : OFF_WC1A + (n + 1) * 128] = Wc1[n, :HID, :]
        wb[:, OFF_WC1B + n * 128 : OFF_WC1B + (n + 1) * 128] = Wc1[n, HID:, :]
        wb[:, OFF_WC2 + n * 16 : OFF_WC2 + (n + 1) * 16] = Wc2[n]
    wb[:, OFF_WK : OFF_WK + 128] = wk_m
    wb[:, OFF_WQ : OFF_WQ + 128] = wq_m
    wb[:, OFF_WV : OFF_WV + 128] = wv_m
    wb[:, OFF_SELBC : OFF_SELBC + 1024] = selbc
    wb[:, OFF_ONES : OFF_ONES + 256] = onesred
    wb[:, OFF_SELDD : OFF_SELDD + 64] = seldd
    wb[:, OFF_SELREP : OFF_SELREP + 256] = selrep

    we2blob = np.zeros((ADIM, 8 * 128), f32)
    for n in range(N):
        we2blob[:, n * 128 : (n + 1) * 128] = We[n, SDIM:, :]

    fb = np.zeros((128, NBF), f32)
    fb[:, FB_BE : FB_BE + 8] = be.T
    fb[:, FB_BS : FB_BS + 8] = bs.T
    fb[:, FB_BV] = bv_m
    fb[:, FB_BC1 : FB_BC1 + 8] = bc1.T

    shared = {"wblob": bf(wb), "we2b": bf(we2blob), "fblob": np.ascontiguousarray(fb, f32)}
    sT_full = bf(states.transpose(0, 2, 1))
    aT_full = bf(actions.transpose(0, 2, 1))

    def core_inputs(c, bsh):
        lo = c * bsh
        return dict(
            shared,
            sT=np.ascontiguousarray(sT_full[:, :, lo : lo + bsh]),
            aT=np.ascontiguousarray(aT_full[:, :, lo : lo + bsh]),
        )

    return core_inputs, acs, np.asarray(bc2, np.float32)


def kernel(**inputs):
    from concourse.bass_utils import run_bass_kernel_spmd

    nc = build_nc(BSH)
    core_inputs, acs, bc2 = host_prep(**inputs)
    in_maps = [core_inputs(c, BSH) for c in range(NCORES)]
    res = run_bass_kernel_spmd(nc, in_maps, list(range(NCORES))).results
    out = np.empty((N, B, 1), np.float32)
    for c in range(NCORES):
        aqT = res[c]["q8T"]  # [16, N, BSH]
        aq = np.ascontiguousarray(aqT.transpose(1, 0, 2))
        sl = slice(c * BSH, (c + 1) * BSH)
        picked = np.take_along_axis(aq, acs[:, None, sl], axis=1)[:, 0, :]
        out[:, sl, 0] = picked + np.take_along_axis(bc2, acs[:, sl], axis=1)
    return out
